# revision 1
# baseline (speedup 1.0000x reference)
"""AdaptiveModulatedConv3d — 8-core TRN2 Bass kernel.

Problem (hardcoded): BS=8, C_IN=C_OUT=64, K=3, STYLE_DIM=512, BANK=4,
D=H=W=32, pad=1, stride=1, f32 in/out.

Sharding: pure data-parallel over batch — each of the 8 NeuronCores gets one
sample, builds its per-sample demodulated conv weights on-device, and runs
its own 3D conv. No collectives.

Per-core conv strategy: the 3x3x3 conv is decomposed into 27 shifted
matmuls (contraction over C_IN=64) accumulating into PSUM. The PE 128x128
array is quadrant-packed: row-groups 0/64 hold two copies of x (bf16), so
two offset-matmuls run concurrently; col-groups 0/64 compute the two
h-halves of one output d-plane in the same PSUM bank. Boundary kernel taps
use narrowed-N matmuls instead of padding, so every DMA is contiguous.
"""

import numpy as np

import concourse.bass as bass
import concourse.tile as tile
from concourse import bacc, mybir
from concourse import bass_utils

F32 = mybir.dt.float32
BF16 = mybir.dt.bfloat16

BS = 8
CI = 64
CO = 64
KK = 3
SD = 512
BANK = 4
D = H = W = 32
EPS = 1e-8
NCORES = 8
DCH = 2  # d-planes per input-convert chunk

_CACHE = {}


def _emit_weight_build(nc, tc, pools, aps):
    """Build WT[128, 27, 64] bf16: WT[ci(+64), kd*9+kh*3+kw, co] =
    demodulated per-sample weight, duplicated on upper 64 partitions."""
    singles = pools["singles"]
    wk, fw, fb, mwt, mb, bankt = (
        aps["wk"], aps["fw"], aps["fb"], aps["mwt"], aps["mb"], aps["bankt"])

    # SBUF copies of the small params
    wk_sb = singles.tile([128, BANK], F32)
    nc.sync.dma_start(out=wk_sb, in_=wk)
    fw_sb = singles.tile([128, BANK, BANK], F32)
    nc.sync.dma_start(out=fw_sb, in_=fw)
    fb_sb = singles.tile([1, BANK], F32)
    nc.sync.dma_start(out=fb_sb, in_=fb)
    mwt_sb = singles.tile([128, BANK, CI], F32)
    nc.sync.dma_start(out=mwt_sb, in_=mwt)
    mb_sb = singles.tile([CI, 1], F32)
    nc.sync.dma_start(out=mb_sb, in_=mb)
    bank_sb = singles.tile([CI, BANK, 27 * CO], BF16)
    nc.sync.dma_start(out=bank_sb, in_=bankt)

    warm = singles.tile([1, 1], F32)
    nc.vector.memset(warm, 0.0)
    nc.scalar.activation(warm, warm, mybir.ActivationFunctionType.Exp)
    ones1 = singles.tile([1, 64], F32)
    nc.vector.memset(ones1, 1.0)
    ones64 = singles.tile([64, 1], BF16)
    nc.vector.memset(ones64, 1.0)

    with tc.tile_pool(name="wpsum", bufs=2, space="PSUM") as wpsum:
        # ---- filter weights: logits = w @ filter_w.T + filter_b ----
        ps_l = wpsum.tile([1, BANK], F32, tag="wps")
        for c in range(4):
            nc.tensor.matmul(ps_l, lhsT=wk_sb[:, c:c + 1], rhs=fw_sb[:, c, :],
                             start=(c == 0), stop=(c == 3))
        logits = singles.tile([1, BANK], F32)
        nc.vector.tensor_add(logits, ps_l, fb_sb)
        # softmax WITHOUT the 1/sum normalization: a uniform scale on the
        # mixed weights cancels exactly through the demodulation (rsqrt of
        # the squared sum; eps perturbation is negligible), so exp(logits)
        # alone suffices and three ops leave the critical chain
        fwt = singles.tile([1, BANK], F32)
        nc.scalar.activation(fwt, logits, mybir.ActivationFunctionType.Exp)

        # ---- mod = w @ mod_w.T + mod_b  -> [ci, 1] ----
        ps_m = wpsum.tile([CI, 1], F32, tag="wps")
        for c in range(4):
            nc.tensor.matmul(ps_m, lhsT=mwt_sb[:, c, :], rhs=wk_sb[:, c:c + 1],
                             start=(c == 0), stop=(c == 3))
        mod_sb = singles.tile([CI, 1], F32)
        nc.vector.tensor_add(mod_sb, ps_m, mb_sb)

        # ---- broadcast fwt across partitions: [64, 4] (stays in PSUM,
        # the mix reads its per-partition scalars straight from there) ----
        ps_fb = wpsum.tile([64, BANK], F32, tag="wps")
        nc.tensor.matmul(ps_fb, lhsT=ones1, rhs=fwt, start=True, stop=True)

        # ---- weighted bank mix + WT = acc*mod, in two koff halves so
        # the conv (which consumes koffs in ascending wave order) starts
        # after the first half; demod commutes with the conv and is
        # applied per-co in the drain scale ----
        acc = singles.tile([CI, 27 * CO], F32)
        WT = singles.tile([128, 27, CO], BF16)
        for (k0, k1) in ((0, 14), (14, 27)):
            f0, f1 = k0 * CO, k1 * CO
            nc.vector.tensor_scalar_mul(acc[:, f0:f1],
                                        bank_sb[:, 0, f0:f1], ps_fb[:, 0:1])
            for n in range(1, 4):
                nc.vector.scalar_tensor_tensor(
                    out=acc[:, f0:f1], in0=bank_sb[:, n, f0:f1],
                    scalar=ps_fb[:, n:n + 1], in1=acc[:, f0:f1],
                    op0=mybir.AluOpType.mult, op1=mybir.AluOpType.add)
            nc.vector.tensor_scalar_mul(
                WT[0:64, k0:k1],
                acc[:, f0:f1].rearrange("p (k c) -> p k c", c=CO),
                mod_sb[:, 0:1])
            nc.sync.dma_start(out=WT[64:128, k0:k1], in_=WT[0:64, k0:k1])

        # ---- demod: rsqrt(sum (acc*mod)^2 per co + eps), squares taken
        # from the bf16 WT on DVE (keeps the ACT queue free for drains) ----
        sq = singles.tile([CI, 27 * CO], BF16)
        nc.vector.tensor_mul(sq, WT[0:64].rearrange("p k c -> p (k c)"),
                             WT[0:64].rearrange("p k c -> p (k c)"))
        partial = singles.tile([1, 4, CO], F32)
        chunks = [(0, 7), (7, 7), (14, 7), (21, 6)]  # koff ranges
        for j, (k0, nk) in enumerate(chunks):
            ps_c = wpsum.tile([1, nk * CO], F32, tag="wps")
            nc.tensor.matmul(ps_c, lhsT=ones64,
                             rhs=sq[:, k0 * CO:(k0 + nk) * CO],
                             start=True, stop=True)
            nc.vector.reduce_sum(
                out=partial[:, j, :],
                in_=ps_c.rearrange("p (k c) -> p c k", c=CO),
                axis=mybir.AxisListType.X)
        dsum = singles.tile([1, CO], F32)
        nc.vector.reduce_sum(out=dsum,
                             in_=partial.rearrange("p j c -> p c j"),
                             axis=mybir.AxisListType.X)
        eps_sb = singles.tile([1, 1], F32)
        nc.vector.memset(eps_sb, EPS)
        sstd = singles.tile([1, CO], F32)
        nc.scalar.activation(sstd, dsum, mybir.ActivationFunctionType.Sqrt,
                             bias=eps_sb[:, 0:1])
        demod = singles.tile([1, CO], F32)
        nc.vector.reciprocal(demod, sstd)

        # ---- transpose demod to per-partition column [co, 1] ----
        ps_t = wpsum.tile([CO, 1], F32, tag="wps")
        nc.tensor.matmul(ps_t, lhsT=demod, rhs=ones1[:, 0:1],
                         start=True, stop=True)
        dmT = singles.tile([CO, 1], F32)
        nc.vector.tensor_copy(dmT, ps_t)

    return WT, dmT


def _conv_offsets(d):
    """Valid (kd, kh, kw) taps for output d-plane d."""
    offs = []
    for kd in range(3):
        if 0 <= d + kd - 1 <= D - 1:
            for kh in range(3):
                for kw in range(3):
                    offs.append((kd, kh, kw))
    return offs




PLANE = (H + 2) * (W + 2)  # 1156, h/w zero-padded plane, flattened
ROWSPLIT = [(0, 11), (11, 11), (22, 10)]  # h-row tiles per d-plane


def _emit_conv(nc, tc, pools, aps, WT, dmT, xbf):
    """3x3x3 conv as 27 shifted matmuls per tile over flattened padded
    planes.

    HW constraints: moving operand = flat contiguous slice; one PSUM
    accumulation group must stay within ONE PE row group. Each of the 4 PE
    quadrants (row group x col group) owns an independent output tile in
    its own PSUM bank; the two x copies feed the two row groups (upper
    copy is stored shifted by +1 element, compensated in the offsets).
    Wave order groups the two rg0 matmuls then the two rg64 matmuls, so
    each LDWEIGHTS can pull ahead under the opposite row group's streams."""
    out_ap = aps["out"]
    osb_pool = pools["osb"]
    tiles = [(d, r0, nr) for d in range(D) for (r0, nr) in ROWSPLIT]
    quads = [(0, 0), (64, 0), (0, 64), (64, 64)]
    with tc.tile_pool(name="cpsum", bufs=8, space="PSUM") as cpsum:
        for ti in range(0, len(tiles), 4):
            group = tiles[ti:ti + 4]
            pss = [cpsum.tile([128, 512], F32, tag="cps", name=f"cps{j}")
                   for j in range(len(group))]
            osbA = osb_pool.tile([128, 2, 374], F32, name="osbA")
            osbs = [osbA[0:64, 0], osbA[0:64, 1],
                    osbA[64:128, 0], osbA[64:128, 1]]
            offs_l = [_conv_offsets(d) for (d, r0, nr) in group]
            nwaves = max(len(o) for o in offs_l)
            for i in range(nwaves):
                for j, (d, r0, nr) in enumerate(group):
                    offs = offs_l[j]
                    if i >= len(offs):
                        continue
                    kd, kh, kw = offs[i]
                    rg, cp = quads[j]
                    koff = kd * 9 + kh * 3 + kw
                    n = nr * 34
                    off = 2 + (d + kd - 1) * PLANE + (r0 + kh) * 34 + kw - 1
                    if rg:
                        off -= 1
                    nc.tensor.matmul(
                        pss[j][cp:cp + 64, 0:n],
                        lhsT=WT[rg:rg + 64, koff, :],
                        rhs=xbf[rg:rg + 64, off:off + n],
                        start=(i == 0), stop=(i == len(offs) - 1))
            # drain: contiguous PSUM -> SBUF copy (fast), junk columns
            # stripped by the strided-source output DMA
            for j, (d, r0, nr) in enumerate(group):
                cp = quads[j][1]
                n = nr * 34
                nc.scalar.mul(osbs[j][:, 0:n], pss[j][cp:cp + 64, 0:n],
                              dmT[:, 0:1])
                osrc = osbs[j][:, 0:n].rearrange(
                    "p (a b) -> p a b", b=34)[:, :, 1:W + 1]
                nc.gpsimd.dma_start(out=out_ap[:, d, r0:r0 + nr, :],
                                    in_=osrc)


def _build():
    nc = bacc.Bacc("TRN2", target_bir_lowering=False, debug=False)
    x = nc.dram_tensor("x", [CI, D, H, W], F32, kind="ExternalInput").ap()
    wk = nc.dram_tensor("wk", [128, BANK], F32, kind="ExternalInput").ap()
    fw = nc.dram_tensor("fw", [128, BANK, BANK], F32,
                        kind="ExternalInput").ap()
    fb = nc.dram_tensor("fb", [1, BANK], F32, kind="ExternalInput").ap()
    mwt = nc.dram_tensor("mwt", [128, BANK, CI], F32,
                         kind="ExternalInput").ap()
    mb = nc.dram_tensor("mb", [CI, 1], F32, kind="ExternalInput").ap()
    bankt = nc.dram_tensor("bankt", [CI, BANK, 27 * CO], BF16,
                           kind="ExternalInput").ap()
    out = nc.dram_tensor("out", [CO, D, H, W], F32, kind="ExternalOutput").ap()
    aps = dict(x=x, wk=wk, fw=fw, fb=fb, mwt=mwt, mb=mb, bankt=bankt, out=out)

    with tile.TileContext(nc) as tc:
        with tc.tile_pool(name="singles", bufs=1) as singles, \
             tc.tile_pool(name="stg", bufs=2) as stg_pool, \
             tc.tile_pool(name="osb", bufs=6) as osb_pool:
            pools = dict(singles=singles, stg=stg_pool, osb=osb_pool)

            WT, dmT = _emit_weight_build(nc, tc, pools, aps)

            # x: f32 HBM -> flat SBUF staging (contiguous DMA) -> bf16
            # cast with strided dest into padded planes. Borders zeroed
            # once by strided memsets. Upper 64 partitions hold the copy
            # shifted by +1 element (offsets compensate).
            xbf = singles.tile([128, 3 + D * PLANE], BF16)
            nc.gpsimd.memset(xbf[:, 0:2], 0.0)
            nc.gpsimd.memset(xbf[:, 2 + D * PLANE:3 + D * PLANE], 0.0)
            nc.gpsimd.memset(xbf[64:128, 1 + D * PLANE:2 + D * PLANE], 0.0)
            pl_all = xbf[:, 2:2 + D * PLANE].rearrange(
                "p (d h w) -> p d h w", h=H + 2, w=W + 2)
            nc.gpsimd.memset(pl_all[:, :, 0, :], 0.0)
            nc.gpsimd.memset(pl_all[:, :, H + 1, :], 0.0)
            nc.gpsimd.memset(pl_all[:, :, :, 0], 0.0)
            nc.gpsimd.memset(pl_all[:, :, :, W + 1], 0.0)
            for s in range(D // DCH):
                stg = stg_pool.tile([CI, DCH, H, W], F32)
                nc.sync.dma_start(out=stg, in_=x[:, s * DCH:(s + 1) * DCH])
                for dd in range(DCH):
                    p = s * DCH + dd
                    b2 = 2 + p * PLANE + (W + 2) + 1
                    dst = xbf[0:64, b2:b2 + H * (W + 2)].rearrange(
                        "p (h w) -> p h w", w=W + 2)[:, :, 0:W]
                    nc.vector.tensor_copy(dst, stg[:, dd])
                    lo, hi = 2 + p * PLANE, 2 + (p + 1) * PLANE
                    nc.sync.dma_start(out=xbf[64:128, lo - 1:hi - 1],
                                      in_=xbf[0:64, lo:hi])

            _emit_conv(nc, tc, pools, aps, WT, dmT, xbf)

    nc.compile()
    return nc


def _shard_inputs(x, w, filter_w, filter_b, mod_w, mod_b, bank):
    """Host-side input marshalling: per-core shards + replicated params in
    the layouts the kernel expects."""
    fw_h = np.ascontiguousarray(
        filter_w.T.reshape(4, 128, BANK).transpose(1, 0, 2), np.float32)
    mwt_h = np.ascontiguousarray(
        mod_w.T.reshape(4, 128, CI).transpose(1, 0, 2), np.float32)
    import ml_dtypes
    bank_h = np.ascontiguousarray(
        bank.reshape(BANK, CO, CI, 27).transpose(2, 0, 3, 1)
        .reshape(CI, BANK, 27 * CO)).astype(ml_dtypes.bfloat16)
    fb_h = np.ascontiguousarray(filter_b.reshape(1, BANK), np.float32)
    mb_h = np.ascontiguousarray(mod_b.reshape(CI, 1), np.float32)
    in_maps = []
    for i in range(NCORES):
        in_maps.append({
            "x": np.ascontiguousarray(x[i], np.float32),
            "wk": np.ascontiguousarray(w[i].reshape(4, 128).T, np.float32),
            "fw": fw_h, "fb": fb_h, "mwt": mwt_h, "mb": mb_h,
            "bankt": bank_h,
        })
    return in_maps


def _run(inputs, trace=False):
    if "nc" not in _CACHE:
        _CACHE["nc"] = _build()
    nc = _CACHE["nc"]
    in_maps = _shard_inputs(**inputs)
    res = bass_utils.run_bass_kernel_spmd(
        nc, in_maps, core_ids=list(range(NCORES)), trace=trace)
    out = np.stack([res.results[i]["out"] for i in range(NCORES)])
    return out.astype(np.float32), res


def kernel(**inputs):
    out, _ = _run(inputs, trace=False)
    return out

